# revision 1
# baseline (speedup 1.0000x reference)
"""Trainium2 Bass kernel for the Chambolle-Pock-style primal/dual stencil loop.

Math (per image, H=W=1024, EPS=0.5, TAU=0.5, 10 iterations):
    u = sigmoid(o/EPS); q = 0
    repeat 10x:
        q  = relu(q - TAU*(vf1*Dy(u) + vf0*Dx(u)))   # forward diffs, zero pad
        Tq = BDy(vf1*q) + BDx(vf0*q)                  # backward diffs, zero pad
        u  = sigmoid((o - Tq)/EPS)
    return (o - Tq)/EPS

Rescaling trick: with qh = 2*sqrt(2)*q, g = vf/sqrt(2) (host-side) and
s = 2(o - Tq), and representing u through t = tanh(s/2) (u = 0.5 + 0.5t, the
0.5s cancel in every stencil difference; zero-padding of u becomes
(-1)-padding of t):
    qh = relu(qh - (g1*Dy(t) + g0*Dx(t)))            # t pads: -1
    s  = o2 - BDy(g1*qh) - BDx(g0*qh)                # o2 = 2*o, pads 0
    t  = tanh(s/2)
and the final output is s itself.  tanh is used instead of sigmoid because its
activation table is ~10x more accurate (4 vs 40 ULP) and the relu makes
isolated pixels chaotic under any per-step rounding noise; everything else is
kept in exact fp32 for the same reason (measured: rel-L2 vs the fp32 jax
reference is ~1e-5, max-abs ~0.028 — the fp32 reference's own fp64 envelope).

Sharding: pure data parallel, one image per NeuronCore (B=8 over 8 cores),
vf0/vf1 broadcast to all cores.

Layout: image row y = 8*p + i -> partition p (0..127), plane i (0..7) in the
free dimension.  A +1 row shift is then a free-dim plane offset for i<7; only
the plane-7 -> next-partition boundary needs a cross-partition move, done with
a tiny SBUF->SBUF DMA into a 9th plane.  Column shifts use guard columns.
The whole working set (6 image buffers, ~200KB/partition) stays SBUF resident,
so HBM traffic is one 12MB load + 4MB store per core.  All elementwise ops run
on VectorE split into plane-halves so relu/tanh (ScalarE) and the boundary
DMAs overlap the VectorE stream.
"""

import numpy as np

import concourse.bacc as bacc
import concourse.mybir as mybir
from concourse.tile import TileContext
from concourse import bass_utils

F32 = mybir.dt.float32
AF = mybir.ActivationFunctionType

B, H, W = 8, 1024, 1024
P = 128          # SBUF partitions
NP = H // P      # planes per partition = 8
WG = W + 1       # plane width incl. one guard column
MAXITER = 10

_CACHE = {}
LAST_RESULTS = None  # BassKernelResults of the most recent run (for test.py)


def _build(reps=1):
    """Build the Bass program.  reps>1 repeats the whole computation (state
    re-initialized each rep, same output) — used only for wall-clock timing
    of the HW kernel when no NTFF profiling is available."""
    nc = bacc.Bacc("TRN2", target_bir_lowering=False, debug=False)

    o2_d = nc.dram_tensor("o2", [H, W], F32, kind="ExternalInput").ap()
    g0_d = nc.dram_tensor("g0", [H, W], F32, kind="ExternalInput").ap()
    g1_d = nc.dram_tensor("g1", [H, W], F32, kind="ExternalInput").ap()
    out_d = nc.dram_tensor("out", [H, W], F32, kind="ExternalOutput").ap()

    # (H, W) -> (p, i, x) with y = 8*p + i
    o2_v = o2_d.rearrange("(p i) x -> p i x", i=NP)
    g0_v = g0_d.rearrange("(p i) x -> p i x", i=NP)
    g1_v = g1_d.rearrange("(p i) x -> p i x", i=NP)
    out_v = out_d.rearrange("(p i) x -> p i x", i=NP)

    v = nc.vector
    act = nc.scalar

    with TileContext(nc) as tc:
        with tc.tile_pool(name="main", bufs=1) as pool:
            o2t = pool.tile([P, NP, W], F32)
            g0t = pool.tile([P, NP, W], F32)
            g1t = pool.tile([P, NP, W], F32)
            qht = pool.tile([P, NP, W], F32)
            # su: planes 0..7 = t/s data (col W = -1 guard for x+1 reads),
            # plane 8 = boundary row t[8p+8, x] (partition 127 stays -1)
            sut = pool.tile([P, NP + 1, WG], F32)
            # tmp: planes 1..8 = a/b scratch at cols 1..W (col 0 = zero guard
            # for x-1 reads), plane 0 = boundary row a[8p-1, x]
            tmpt = pool.tile([P, NP + 1, WG], F32)

            halves = [(0, NP // 2), (NP // 2, NP)]

            def u_(lo, hi):
                return sut[:, lo:hi, 0:W]

            def unr(lo, hi):   # t[y+1, x] (plane 8 = boundary)
                return sut[:, lo + 1 : hi + 1, 0:W]

            def unc(lo, hi):   # t[y, x+1] (col W = -1 guard)
                return sut[:, lo:hi, 1 : W + 1]

            def t_(lo, hi):
                return tmpt[:, lo + 1 : hi + 1, 1 : W + 1]

            def tpr(lo, hi):   # a[y-1, x] (plane 0 = boundary)
                return tmpt[:, lo:hi, 1 : W + 1]

            def tpc(lo, hi):   # b[y, x-1] (col 0 = zero guard)
                return tmpt[:, lo + 1 : hi + 1, 0:W]

            def o2_(lo, hi):
                return o2t[:, lo:hi, :]

            def g0_(lo, hi):
                return g0t[:, lo:hi, :]

            def g1_(lo, hi):
                return g1t[:, lo:hi, :]

            def qh_(lo, hi):
                return qht[:, lo:hi, :]

            # --- setup ---
            # t-state guards are -1 (tanh representation of u=0 padding).
            v.memset(sut[:, :, :], -1.0)
            v.memset(tmpt[:, :, :], 0.0)  # zero guards + a-boundary row 0
            nc.sync.dma_start(out=o2t[:, :, :], in_=o2_v)
            nc.sync.dma_start(out=g0t[:, :, :], in_=g0_v)
            nc.sync.dma_start(out=g1t[:, :, :], in_=g1_v)

            def dma_ushift():
                # su[p, 8, x] = t[8p+8, x] = su[p+1, 0, x]; row 127 stays -1
                nc.sync.dma_start(
                    out=sut[0 : P - 1, NP, 0:W], in_=sut[1:P, 0, 0:W]
                )

            def dma_ashift():
                # tmp[p, 0, c] = a[8p-1] = tmp[p-1, 8, c]; row 0 stays 0
                nc.sync.dma_start(
                    out=tmpt[1:P, 0, 1 : W + 1], in_=tmpt[0 : P - 1, NP, 1 : W + 1]
                )

            for _rep in range(reps):
                if reps > 1:
                    v.memset(sut[:, :, :], -1.0)
                v.memset(qht[:, :, :], 0.0)
                for lo, hi in halves:
                    act.activation(u_(lo, hi), o2_(lo, hi), AF.Tanh, scale=0.5)
                dma_ushift()

                for it in range(MAXITER):
                    last = it == MAXITER - 1
                    # dual: qh = relu(qh - g1*Dy(t) - g0*Dx(t))
                    for lo, hi in halves:
                        v.tensor_sub(t_(lo, hi), unr(lo, hi), u_(lo, hi))
                        v.tensor_mul(t_(lo, hi), t_(lo, hi), g1_(lo, hi))
                        v.tensor_sub(qh_(lo, hi), qh_(lo, hi), t_(lo, hi))
                    for lo, hi in halves:
                        v.tensor_sub(t_(lo, hi), unc(lo, hi), u_(lo, hi))
                        v.tensor_mul(t_(lo, hi), t_(lo, hi), g0_(lo, hi))
                        v.tensor_sub(qh_(lo, hi), qh_(lo, hi), t_(lo, hi))
                        act.activation(qh_(lo, hi), qh_(lo, hi), AF.Relu)
                    # primal: s = o2 - (a-a_pr) - (b-b_pc), a = g1*qh, b = g0*qh
                    # upper a-half first so the boundary-row DMA fires early
                    v.tensor_mul(t_(*halves[1]), g1_(*halves[1]), qh_(*halves[1]))
                    dma_ashift()
                    v.tensor_mul(t_(*halves[0]), g1_(*halves[0]), qh_(*halves[0]))
                    for lo, hi in halves:
                        v.tensor_sub(u_(lo, hi), o2_(lo, hi), t_(lo, hi))
                        v.tensor_add(u_(lo, hi), u_(lo, hi), tpr(lo, hi))
                    for lo, hi in halves:
                        v.tensor_mul(t_(lo, hi), g0_(lo, hi), qh_(lo, hi))
                        v.tensor_sub(u_(lo, hi), u_(lo, hi), t_(lo, hi))
                        v.tensor_add(u_(lo, hi), u_(lo, hi), tpc(lo, hi))
                        if not last:
                            act.activation(
                                u_(lo, hi), u_(lo, hi), AF.Tanh, scale=0.5
                            )
                            if lo == 0:
                                dma_ushift()

            nc.sync.dma_start(out=out_v, in_=sut[:, 0:NP, 0:W])

    nc.compile()
    return nc


def kernel(o, vector_field, nabla_w, div_w):
    global LAST_RESULTS
    if "nc" not in _CACHE:
        _CACHE["nc"] = _build()
    nc = _CACHE["nc"]

    o2 = np.ascontiguousarray(2.0 * np.asarray(o, dtype=np.float32)[:, 0])
    vf = np.asarray(vector_field, dtype=np.float32)
    s = np.float32(1.0 / np.sqrt(2.0))
    g0 = np.ascontiguousarray(vf[:, :, 0] * s)
    g1 = np.ascontiguousarray(vf[:, :, 1] * s)

    in_maps = [{"o2": o2[b], "g0": g0, "g1": g1} for b in range(B)]
    res = bass_utils.run_bass_kernel_spmd(nc, in_maps, core_ids=list(range(B)))
    LAST_RESULTS = res
    return np.stack([r["out"] for r in res.results]).astype(np.float32)



# revision 2
# speedup vs baseline: 2.1456x; 2.1456x over previous
"""Trainium2 Bass kernel for the Chambolle-Pock-style primal/dual stencil loop.

Math (per image, H=W=1024, EPS=0.5, TAU=0.5, 10 iterations):
    u = sigmoid(o/EPS); q = 0
    repeat 10x:
        q  = relu(q - TAU*(vf1*Dy(u) + vf0*Dx(u)))   # forward diffs, zero pad
        Tq = BDy(vf1*q) + BDx(vf0*q)                  # backward diffs, zero pad
        u  = sigmoid((o - Tq)/EPS)
    return (o - Tq)/EPS

Rescaling trick (see kernel_baseline_f32.py): with qh = 2*sqrt(2)*q,
g = vf/sqrt(2), o2 = 2*o, s = 2(o - Tq) and t = tanh(s/2) (u = 0.5+0.5t,
zero-padding of u becomes (-1)-padding of t):
    qh = relu(qh - (g1*Dy(t) + g0*Dx(t)))            # t pads: -1
    s  = o2 - BDy(g1*qh) - BDx(g0*qh)                # pads 0
    t  = tanh(s/2)
and the final output is s itself.

This version runs the whole loop in FP16 (validated: rel-L2 vs the fp32 jax
reference ~8e-3, under the 2e-2 gate; the error comes from early relu
decision-boundary bifurcations, not accumulation).  FP16 engages the DVE
2x_1p mode (0.52 ns/elem vs 1.04 fp32), and every tensor-tensor op is
emitted as a DVE instruction on columns [0, XS) plus a GpSimd (Pool engine)
instruction on columns [XS, W) running concurrently (Pool TT runs at 0.42
of its 0.83 ns/elem roofline, so the 816/208 column split balances the two
engines).  relu/tanh run on the Scalar (Activation) engine in plane halves
so they overlap the vector stream.

Sharding: pure data parallel, one image per NeuronCore (B=8 over 8 cores),
g0/g1 broadcast to all cores.

Layout: image row y = 8*p + i -> partition p (0..127), plane i (0..7) in the
free dimension.  A +1 row shift is a free-dim plane offset for i<7; only the
plane-7 -> next-partition boundary needs a tiny SBUF->SBUF DMA into a 9th
plane.  Column shifts use guard columns.
"""

import numpy as np

import concourse.bacc as bacc
import concourse.mybir as mybir
from concourse.tile import TileContext
from concourse import bass_utils

F32 = mybir.dt.float32
F16 = mybir.dt.float16
AF = mybir.ActivationFunctionType

B, H, W = 8, 1024, 1024
P = 128          # SBUF partitions
NP = H // P      # planes per partition = 8
WG = W + 2       # plane width incl. one guard column (+1 pad to even)
XS = 816         # DVE handles cols [0, XS), Pool cols [XS, W)
MAXITER = 10
HALVES = [(0, NP // 2), (NP // 2, NP)]

_CACHE = {}
LAST_RESULTS = None  # BassKernelResults of the most recent run (for test.py)


def _build(reps=1):
    nc = bacc.Bacc("TRN2", target_bir_lowering=False, debug=False)

    o2_d = nc.dram_tensor("o2", [H, W], F16, kind="ExternalInput").ap()
    g0_d = nc.dram_tensor("g0", [H, W], F16, kind="ExternalInput").ap()
    g1_d = nc.dram_tensor("g1", [H, W], F16, kind="ExternalInput").ap()
    out_d = nc.dram_tensor("out", [H, W], F16, kind="ExternalOutput").ap()

    # (H, W) -> (p, i, x) with y = 8*p + i
    o2_v = o2_d.rearrange("(p i) x -> p i x", i=NP)
    g0_v = g0_d.rearrange("(p i) x -> p i x", i=NP)
    g1_v = g1_d.rearrange("(p i) x -> p i x", i=NP)
    out_v = out_d.rearrange("(p i) x -> p i x", i=NP)

    v = nc.vector
    gp = nc.gpsimd
    act = nc.scalar

    with TileContext(nc) as tc:
        with tc.tile_pool(name="main", bufs=1) as pool:
            o2t = pool.tile([P, NP, W], F16)
            g0t = pool.tile([P, NP, W], F16)
            g1t = pool.tile([P, NP, W], F16)
            qht = pool.tile([P, NP, W], F16)
            # su: planes 0..7 = t/s data (col W = -1 guard for x+1 reads),
            # plane 8 = boundary row t[8p+8, x] (partition 127 stays -1)
            sut = pool.tile([P, NP + 1, WG], F16)
            # tmp: planes 1..8 = a/b scratch at cols 1..W (col 0 = zero guard
            # for x-1 reads), plane 0 = boundary row a[8p-1, x]
            tmpt = pool.tile([P, NP + 1, WG], F16)

            # column ranges per engine: (engine, c0, c1)
            ENG = [(v, 0, XS), (gp, XS, W)]

            def u_(lo, hi, c0, c1):
                return sut[:, lo:hi, c0:c1]

            def unr(lo, hi, c0, c1):   # t[y+1, x] (plane 8 = boundary)
                return sut[:, lo + 1 : hi + 1, c0:c1]

            def unc(lo, hi, c0, c1):   # t[y, x+1] (col W = -1 guard)
                return sut[:, lo:hi, c0 + 1 : c1 + 1]

            def t_(lo, hi, c0, c1):
                return tmpt[:, lo + 1 : hi + 1, c0 + 1 : c1 + 1]

            def tpr(lo, hi, c0, c1):   # a[y-1, x] (plane 0 = boundary)
                return tmpt[:, lo:hi, c0 + 1 : c1 + 1]

            def tpc(lo, hi, c0, c1):   # b[y, x-1] (col 0 = zero guard)
                return tmpt[:, lo + 1 : hi + 1, c0:c1]

            def o2_(lo, hi, c0, c1):
                return o2t[:, lo:hi, c0:c1]

            def g0_(lo, hi, c0, c1):
                return g0t[:, lo:hi, c0:c1]

            def g1_(lo, hi, c0, c1):
                return g1t[:, lo:hi, c0:c1]

            def qh_(lo, hi, c0, c1):
                return qht[:, lo:hi, c0:c1]

            def emit(op_name, lo, hi, out_f, a_f, b_f):
                """Emit one logical TT op on planes [lo,hi) as a DVE/Pool
                column-split pair."""
                for eng, c0, c1 in ENG:
                    op = getattr(eng, op_name)
                    op(out_f(lo, hi, c0, c1), a_f(lo, hi, c0, c1),
                       b_f(lo, hi, c0, c1))

            # --- setup ---
            v.memset(sut[:, :, :], -1.0)
            v.memset(tmpt[:, :, :], 0.0)  # zero guards + a-boundary row 0
            nc.sync.dma_start(out=o2t[:, :, :], in_=o2_v)
            nc.sync.dma_start(out=g0t[:, :, :], in_=g0_v)
            nc.sync.dma_start(out=g1t[:, :, :], in_=g1_v)

            def dma_ushift():
                # su[p, 8, x] = t[8p+8, x] = su[p+1, 0, x]; row 127 stays -1
                nc.sync.dma_start(
                    out=sut[0 : P - 1, NP, 0:W], in_=sut[1:P, 0, 0:W]
                )

            def dma_ashift():
                # tmp[p, 0, c] = a[8p-1] = tmp[p-1, 8, c]; row 0 stays 0
                nc.sync.dma_start(
                    out=tmpt[1:P, 0, 1 : W + 1], in_=tmpt[0 : P - 1, NP, 1 : W + 1]
                )

            for _rep in range(reps):
                if reps > 1:
                    v.memset(sut[:, :, :], -1.0)
                v.memset(qht[:, :, :], 0.0)
                for lo, hi in HALVES:
                    act.activation(
                        u_(lo, hi, 0, W), o2_(lo, hi, 0, W), AF.Tanh, scale=0.5
                    )
                dma_ushift()

                for it in range(MAXITER):
                    last = it == MAXITER - 1
                    # dual: qh = relu(qh - g1*Dy(t) - g0*Dx(t))
                    for lo, hi in HALVES:
                        emit("tensor_sub", lo, hi, t_, unr, u_)
                        emit("tensor_mul", lo, hi, t_, t_, g1_)
                        emit("tensor_sub", lo, hi, qh_, qh_, t_)
                    for lo, hi in HALVES:
                        emit("tensor_sub", lo, hi, t_, unc, u_)
                        emit("tensor_mul", lo, hi, t_, t_, g0_)
                        emit("tensor_sub", lo, hi, qh_, qh_, t_)
                        act.activation(
                            qh_(lo, hi, 0, W), qh_(lo, hi, 0, W), AF.Relu
                        )
                    # primal: s = o2 - (a-a_pr) - (b-b_pc), a = g1*qh, b = g0*qh
                    # upper a-half first so the boundary-row DMA fires early
                    emit("tensor_mul", *HALVES[1], t_, g1_, qh_)
                    dma_ashift()
                    emit("tensor_mul", *HALVES[0], t_, g1_, qh_)
                    for lo, hi in HALVES:
                        emit("tensor_sub", lo, hi, u_, o2_, t_)
                        emit("tensor_add", lo, hi, u_, u_, tpr)
                    for lo, hi in HALVES:
                        emit("tensor_mul", lo, hi, t_, g0_, qh_)
                        emit("tensor_sub", lo, hi, u_, u_, t_)
                        emit("tensor_add", lo, hi, u_, u_, tpc)
                        if not last:
                            act.activation(
                                u_(lo, hi, 0, W), u_(lo, hi, 0, W),
                                AF.Tanh, scale=0.5,
                            )
                            if lo == 0:
                                dma_ushift()

            nc.sync.dma_start(out=out_v, in_=sut[:, 0:NP, 0:W])

    nc.compile()
    return nc


def kernel(o, vector_field, nabla_w, div_w):
    global LAST_RESULTS
    if "nc" not in _CACHE:
        _CACHE["nc"] = _build()
    nc = _CACHE["nc"]

    o2 = np.ascontiguousarray(
        (2.0 * np.asarray(o, dtype=np.float32)[:, 0]).astype(np.float16)
    )
    vf = np.asarray(vector_field, dtype=np.float32)
    s = np.float32(1.0 / np.sqrt(2.0))
    g0 = np.ascontiguousarray((vf[:, :, 0] * s).astype(np.float16))
    g1 = np.ascontiguousarray((vf[:, :, 1] * s).astype(np.float16))

    in_maps = [{"o2": o2[b], "g0": g0, "g1": g1} for b in range(B)]
    res = bass_utils.run_bass_kernel_spmd(nc, in_maps, core_ids=list(range(B)))
    LAST_RESULTS = res
    return np.stack([r["out"] for r in res.results]).astype(np.float32)


# revision 5
# speedup vs baseline: 2.8114x; 1.3103x over previous
"""Trainium2 Bass kernel for the Chambolle-Pock-style primal/dual stencil loop.

Math (per image, H=W=1024, EPS=0.5, TAU=0.5, 10 iterations):
    u = sigmoid(o/EPS); q = 0
    repeat 10x:
        q  = relu(q - TAU*(vf1*Dy(u) + vf0*Dx(u)))   # forward diffs, zero pad
        Tq = BDy(vf1*q) + BDx(vf0*q)                  # backward diffs, zero pad
        u  = sigmoid((o - Tq)/EPS)
    return (o - Tq)/EPS

Rescaling (see kernel_baseline_f32.py): with qh = 2*sqrt(2)*q, g = vf/sqrt(2),
o2 = 2*o, s = 2(o - Tq), t = tanh(s/2)  (u = 0.5 + 0.5*t; zero-padding of u
becomes (-1)-padding of t):
    qh = relu(qh - g1*(St - t) - g0*(Rt - t))        # S: y+1 shift, R: x+1
    s  = o2 - (a - Sa) - (b - Rb),  a = g1*qh, b = g0*qh   # backward diffs
    t  = tanh(s/2)
and the final output is s.

Engine split (all state fp16; validated rel-L2 vs fp32 jax reference ~8e-3,
under the 2e-2 gate — the error is early relu decision-boundary noise, not
accumulation):
  - 8 tensor-tensor ops/iter run column-split on DVE (cols 0:816, fp16
    2x_1p mode) + GpSimd/Pool (cols 816:1024, 0.42-efficiency TT):
      d1 = St - t; t1 = ng1*d1; d2 = Rt - t; t2 = ng0*d2
      na = ng1*qh; nb = ng0*qh; da = na - Sna; db = nb - Rnb
    (ng = -g host-side, so every PE pass below uses +identity weights)
  - the 6 remaining adds/iter run on the idle TensorEngine as identity
    matmuls accumulating in PSUM (fp32 — better numerics than fp16 adds):
      psum1 = I*qh + I*t1 + I*t2        -> ScalarE relu-drain -> qh (fp16)
      psum2 = I*o2 + I*da + I*db        -> ScalarE tanh-drain -> t  (fp16)
  - last iteration: psum2 is DMA'd to DRAM directly as the fp32 output.

Sharding: pure data parallel, one image per NeuronCore (B=8 over 8 cores),
g0/g1 broadcast.

Layout: image row y = 8*p + i -> partition p (0..127), plane i (0..7) in the
free dim.  Row shifts are free-dim plane offsets; only the plane7 ->
next-partition boundary needs a tiny SBUF->SBUF DMA (9th plane / 0th plane).
Column shifts use guard columns.  PSUM chunk = 1 plane ([128,1024] fp32 =
2 banks); dual + primal pools double-buffered = all 8 banks.
"""

import numpy as np

import concourse.bacc as bacc
import concourse.mybir as mybir
from concourse.tile import TileContext
from concourse import bass_utils

F32 = mybir.dt.float32
F16 = mybir.dt.float16
AF = mybir.ActivationFunctionType

B, H, W = 8, 1024, 1024
P = 128          # SBUF partitions
NP = H // P      # planes per partition = 8
WG = W + 2       # plane width incl. one guard column (+1 pad to even)
XS = 816         # DVE handles cols [0, XS), Pool cols [XS, W)
NB = 2           # planes per TT block
MAXITER = 10

_CACHE = {}
LAST_RESULTS = None  # BassKernelResults of the most recent run (for test.py)


def _build(reps=1):
    nc = bacc.Bacc("TRN2", target_bir_lowering=False, debug=False)

    o2_d = nc.dram_tensor("o2", [H, W], F16, kind="ExternalInput").ap()
    g0_d = nc.dram_tensor("ng0", [H, W], F16, kind="ExternalInput").ap()
    g1_d = nc.dram_tensor("ng1", [H, W], F16, kind="ExternalInput").ap()
    eye_d = nc.dram_tensor("eye", [P, P], F16, kind="ExternalInput").ap()
    out_d = nc.dram_tensor("out", [H, W], F32, kind="ExternalOutput").ap()

    # (H, W) -> (p, i, x) with y = 8*p + i
    o2_v = o2_d.rearrange("(p i) x -> p i x", i=NP)
    g0_v = g0_d.rearrange("(p i) x -> p i x", i=NP)
    g1_v = g1_d.rearrange("(p i) x -> p i x", i=NP)
    out_v = out_d.rearrange("(p i) x -> p i x", i=NP)

    v = nc.vector
    gp = nc.gpsimd
    act = nc.scalar
    pe = nc.tensor

    with TileContext(nc) as tc:
        with (
            tc.tile_pool(name="main", bufs=1) as pool,
            tc.tile_pool(name="ps1", bufs=2, space="PSUM") as psp1,
            tc.tile_pool(name="ps2", bufs=2, space="PSUM") as psp2,
        ):
            o2t = pool.tile([P, NP, W], F16)
            ng0t = pool.tile([P, NP, W], F16)
            ng1t = pool.tile([P, NP, W], F16)
            qht = pool.tile([P, NP, W], F16)
            eyet = pool.tile([P, P], F16)
            # sut: planes 0..7 = t data (col W = -1 guard for x+1 reads),
            # plane 8 = boundary row t[8p+8, x] (partition 127 stays -1)
            sut = pool.tile([P, NP + 1, WG], F16)
            # d1/d2: diff scratch, overwritten in place by t1/t2
            d1t = pool.tile([P, NP, W], F16)
            d2t = pool.tile([P, NP, W], F16)
            # na: planes 1..8 = -a data, plane 0 = boundary row -a[8p-1, x]
            nat = pool.tile([P, NP + 1, W], F16)
            # nb: cols 1..W = -b data, col 0 = zero guard for x-1 reads
            nbt = pool.tile([P, NP, WG], F16)
            dat = pool.tile([P, NP, W], F16)
            dbt = pool.tile([P, NP, W], F16)
            # fp32 staging for the last-iteration output drain (PSUM can't
            # DMA to DRAM directly)
            outst = pool.tile([P, 2, W], F32)

            ENG = [(v, 0, XS), (gp, XS, W)]
            BLOCKS = [(b * NB, (b + 1) * NB) for b in range(NP // NB)]

            def u_(lo, hi, c0, c1):
                return sut[:, lo:hi, c0:c1]

            def unr(lo, hi, c0, c1):   # t[y+1, x] (plane 8 = boundary)
                return sut[:, lo + 1 : hi + 1, c0:c1]

            def unc(lo, hi, c0, c1):   # t[y, x+1] (col W = -1 guard)
                return sut[:, lo:hi, c0 + 1 : c1 + 1]

            def mk(tile):
                def f(lo, hi, c0, c1):
                    return tile[:, lo:hi, c0:c1]
                return f

            o2_, ng0_, ng1_, qh_ = mk(o2t), mk(ng0t), mk(ng1t), mk(qht)
            d1_, d2_, da_, db_ = mk(d1t), mk(d2t), mk(dat), mk(dbt)

            def na_(lo, hi, c0, c1):     # -a data (planes 1..8)
                return nat[:, lo + 1 : hi + 1, c0:c1]

            def napr(lo, hi, c0, c1):    # -a[y-1, x] (plane 0 = boundary)
                return nat[:, lo:hi, c0:c1]

            def nb_(lo, hi, c0, c1):     # -b data (cols 1..W)
                return nbt[:, lo:hi, c0 + 1 : c1 + 1]

            def nbpc(lo, hi, c0, c1):    # -b[y, x-1] (col 0 = zero guard)
                return nbt[:, lo:hi, c0:c1]

            def emit(op_name, lo, hi, out_f, a_f, b_f):
                for eng, c0, c1 in ENG:
                    getattr(eng, op_name)(
                        out_f(lo, hi, c0, c1), a_f(lo, hi, c0, c1),
                        b_f(lo, hi, c0, c1))

            # --- setup ---
            v.memset(sut[:, :, :], -1.0)
            v.memset(nat[:, :, :], 0.0)   # zero guard plane 0 (partition 0)
            v.memset(nbt[:, :, :], 0.0)   # zero guard col 0
            nc.sync.dma_start(out=o2t[:, :, :], in_=o2_v)
            nc.sync.dma_start(out=ng0t[:, :, :], in_=g0_v)
            nc.sync.dma_start(out=ng1t[:, :, :], in_=g1_v)
            nc.sync.dma_start(out=eyet[:, :], in_=eye_d)

            def dma_ushift():
                # su[p, 8, x] = t[8p+8, x] = su[p+1, 0, x]; row 127 stays -1
                nc.sync.dma_start(
                    out=sut[0 : P - 1, NP, 0:W], in_=sut[1:P, 0, 0:W]
                )

            def dma_ashift():
                # na[p, 0, x] = -a[8p-1] = na[p-1, 8, x]; row 0 stays 0
                nc.sync.dma_start(
                    out=nat[1:P, 0, 0:W], in_=nat[0 : P - 1, NP, 0:W]
                )

            def accum(ps, srcs, start0):
                """Accumulate identity passes of [P, W] chunk APs into psum."""
                for k, src in enumerate(srcs):
                    for h in range(W // 512):
                        pe.matmul(
                            ps[:, h * 512 : (h + 1) * 512],
                            eyet[:, :],
                            src[:, h * 512 : (h + 1) * 512],
                            start=(k == 0),
                            stop=(k == len(srcs) - 1),
                        )

            for _rep in range(reps):
                if reps > 1:
                    v.memset(sut[:, :, :], -1.0)
            # t0 = tanh(o2/2); qh0 = 0
                v.memset(qht[:, :, :], 0.0)
                for lo, hi in [(0, NP // 2), (NP // 2, NP)]:
                    act.activation(
                        sut[:, lo:hi, 0:W], o2t[:, lo:hi, :], AF.Tanh, scale=0.5
                    )
                dma_ushift()

                for it in range(MAXITER):
                    last = it == MAXITER - 1
                    # --- dual TT stage ---
                    for lo, hi in BLOCKS:
                        emit("tensor_sub", lo, hi, d1_, unr, u_)
                        emit("tensor_mul", lo, hi, d1_, d1_, ng1_)
                        emit("tensor_sub", lo, hi, d2_, unc, u_)
                        emit("tensor_mul", lo, hi, d2_, d2_, ng0_)
                    # --- dual PE + relu drains (chunk = 1 plane) ---
                    for i in range(NP):
                        ps1 = psp1.tile([P, W], F32, name="ps1")
                        accum(ps1, [qht[:, i, :], d1t[:, i, :], d2t[:, i, :]],
                              True)
                        act.activation(qht[:, i, :], ps1[:, :], AF.Relu)
                    # --- primal TT stage ---
                    for lo, hi in BLOCKS:
                        emit("tensor_mul", lo, hi, na_, ng1_, qh_)
                        emit("tensor_mul", lo, hi, nb_, ng0_, qh_)
                        emit("tensor_sub", lo, hi, db_, nb_, nbpc)
                    dma_ashift()
                    for lo, hi in BLOCKS:
                        emit("tensor_sub", lo, hi, da_, na_, napr)
                    # --- primal PE + tanh drains / output ---
                    for i in range(NP):
                        ps2 = psp2.tile([P, W], F32, name="ps2")
                        accum(ps2, [o2t[:, i, :], dat[:, i, :], dbt[:, i, :]],
                              True)
                        if last and reps == 1:
                            st = outst[:, i % 2, :]
                            act.activation(st, ps2[:, :], AF.Copy)
                            nc.sync.dma_start(out=out_v[:, i, :], in_=st)
                        else:
                            act.activation(
                                sut[:, i, 0:W], ps2[:, :], AF.Tanh, scale=0.5
                            )
                            if i == 0:
                                dma_ushift()

    nc.compile()
    return nc


def kernel(o, vector_field, nabla_w, div_w):
    global LAST_RESULTS
    if "nc" not in _CACHE:
        _CACHE["nc"] = _build()
    nc = _CACHE["nc"]

    o2 = np.ascontiguousarray(
        (2.0 * np.asarray(o, dtype=np.float32)[:, 0]).astype(np.float16)
    )
    vf = np.asarray(vector_field, dtype=np.float32)
    s = np.float32(-1.0 / np.sqrt(2.0))
    ng0 = np.ascontiguousarray((vf[:, :, 0] * s).astype(np.float16))
    ng1 = np.ascontiguousarray((vf[:, :, 1] * s).astype(np.float16))
    eye = np.eye(P, dtype=np.float16)

    in_maps = [
        {"o2": o2[b], "ng0": ng0, "ng1": ng1, "eye": eye} for b in range(B)
    ]
    res = bass_utils.run_bass_kernel_spmd(nc, in_maps, core_ids=list(range(B)))
    LAST_RESULTS = res
    return np.stack([r["out"] for r in res.results]).astype(np.float32)


# revision 7
# speedup vs baseline: 3.8357x; 1.3643x over previous
"""Trainium2 Bass kernel for the Chambolle-Pock-style primal/dual stencil loop.

Math (per image, H=W=1024, EPS=0.5, TAU=0.5, 10 iterations):
    u = sigmoid(o/EPS); q = 0
    repeat 10x:
        q  = relu(q - TAU*(vf1*Dy(u) + vf0*Dx(u)))   # forward diffs, zero pad
        Tq = BDy(vf1*q) + BDx(vf0*q)                  # backward diffs, zero pad
        u  = sigmoid((o - Tq)/EPS)
    return (o - Tq)/EPS

Rescaling: with qh = 2*sqrt(2)*q, g = vf/sqrt(2), o2 = 2*o, s = 2(o - Tq),
t = tanh(s/2)  (u = 0.5 + 0.5*t; zero-padding of u becomes (-1)-padding of t):
    qh = relu(qh - g1*(St - t) - g0*(Rt - t))        # S: y+1 shift, R: x+1
    s  = o2 - (a - Sa) - (b - Rb),  a = g1*qh, b = g0*qh   # backward diffs
    t  = tanh(s/2)
and the final output is s.

Three-engine split (all state fp16; validated rel-L2 vs the fp32 jax
reference ~9e-3, under the 2e-2 gate — the error is early relu
decision-boundary noise, not accumulation):
  - 7 tensor-tensor ops/iter run column-split on DVE (cols 0:840, fp16
    2x_1p mode, 0.52 ns/elem) + GpSimd/Pool (cols 840:1024, TT at 0.42 of
    0.83 ns/elem); the split latency-balances the two engines per op:
      d1 = St - t;  t1 = ng1*d1;  d2 = Rt - t;  t2 = ng0*d2
      na = ng1*qh;  nb = ng0*qh;  db = nb - Rnb          (ng = -g, host-side)
  - the remaining 7 adds/iter run on the otherwise-idle TensorEngine as
    +/-identity matmuls accumulating in PSUM (fp32 accumulation - better
    numerics than fp16 adds), one plane ([128,1024] fp32 = 2 PSUM banks)
    per chunk, double-buffered dual + primal pools = all 8 banks:
      psum1 = I*qh + I*t1 + I*t2                 -> ScalarE relu -> qh (fp16)
      psum2 = I*o2 + I*na - I*Sna + I*db         -> ScalarE tanh(x/2) -> t
    (Sna is the plane-shifted read of na, so the "da" diff costs a PE pass
    instead of a DVE op; db keeps the DVE form - that balance measured best.)
  - last iteration: psum2 chunks are Copy-drained to fp16 and DMA'd out on
    alternating queues; the host upcasts to fp32.

Boundary handling: image row y = 8*p + i -> partition p (0..127), plane i
(0..7) in the free dim.  Row shifts are free-dim plane offsets; only the
plane7 -> next-partition boundary crosses partitions, via a tiny SBUF->SBUF
DMA per iteration (t: plane 8 of sut; -a: plane 0 of nat, with a dedicated
plane-7 row multiply emitted first so the DMA fires early).  Column shifts
use guard columns (sut col W = -1; nbt col 0 = 0).  Only guard regions are
memset; interiors are fully overwritten (and the iteration-0 dual skips the
I*qh pass since qh0 = 0, so qht needs no init at all).

Sharding: pure data parallel, one image per NeuronCore (B=8 over 8 cores),
ng0/ng1 broadcast.
"""

import numpy as np

import concourse.bacc as bacc
import concourse.mybir as mybir
from concourse.tile import TileContext
from concourse import bass_utils

F32 = mybir.dt.float32
F16 = mybir.dt.float16
AF = mybir.ActivationFunctionType

B, H, W = 8, 1024, 1024
P = 128          # SBUF partitions
NP = H // P      # planes per partition = 8
WG = W + 2       # plane width incl. one guard column (+1 pad to even)
XS = 840         # DVE handles cols [0, XS), Pool cols [XS, W)
NB = 2           # planes per tensor-tensor block
MAXITER = 10

_CACHE = {}
LAST_RESULTS = None  # BassKernelResults of the most recent run (for test.py)


def _build(reps=1):
    nc = bacc.Bacc("TRN2", target_bir_lowering=False, debug=False)

    o2_d = nc.dram_tensor("o2", [H, W], F16, kind="ExternalInput").ap()
    g0_d = nc.dram_tensor("ng0", [H, W], F16, kind="ExternalInput").ap()
    g1_d = nc.dram_tensor("ng1", [H, W], F16, kind="ExternalInput").ap()
    eye_d = nc.dram_tensor("eye", [P, P], F16, kind="ExternalInput").ap()
    neye_d = nc.dram_tensor("neye", [P, P], F16, kind="ExternalInput").ap()
    out_d = nc.dram_tensor("out", [H, W], F16, kind="ExternalOutput").ap()

    # (H, W) -> (p, i, x) with y = 8*p + i
    o2_v = o2_d.rearrange("(p i) x -> p i x", i=NP)
    g0_v = g0_d.rearrange("(p i) x -> p i x", i=NP)
    g1_v = g1_d.rearrange("(p i) x -> p i x", i=NP)
    out_v = out_d.rearrange("(p i) x -> p i x", i=NP)

    v = nc.vector
    gp = nc.gpsimd
    act = nc.scalar
    pe = nc.tensor

    with TileContext(nc) as tc:
        with (
            tc.tile_pool(name="main", bufs=1) as pool,
            tc.tile_pool(name="ps1", bufs=2, space="PSUM") as psp1,
            tc.tile_pool(name="ps2", bufs=2, space="PSUM") as psp2,
        ):
            o2t = pool.tile([P, NP, W], F16)
            ng0t = pool.tile([P, NP, W], F16)
            ng1t = pool.tile([P, NP, W], F16)
            qht = pool.tile([P, NP, W], F16)
            eyet = pool.tile([P, P], F16)
            neyet = pool.tile([P, P], F16)
            # sut: planes 0..7 = t data (col W = -1 guard for x+1 reads),
            # plane 8 = boundary row t[8p+8, x] (partition 127 stays -1)
            sut = pool.tile([P, NP + 1, WG], F16)
            # d1/d2: diff scratch, overwritten in place by t1/t2
            d1t = pool.tile([P, NP, W], F16)
            d2t = pool.tile([P, NP, W], F16)
            # na: planes 1..8 = -a data, plane 0 = boundary row -a[8p-1, x]
            nat = pool.tile([P, NP + 1, W], F16)
            # nb: cols 1..W = -b data, col 0 = zero guard for x-1 reads
            nbt = pool.tile([P, NP, WG], F16)
            dbt = pool.tile([P, NP, W], F16)
            outst = pool.tile([P, 4, W], F16)   # last-iter drain staging

            ENG = [(v, 0, XS), (gp, XS, W)]
            BLOCKS = [(b * NB, (b + 1) * NB) for b in range(NP // NB)]

            def u_(lo, hi, c0, c1):
                return sut[:, lo:hi, c0:c1]

            def unr(lo, hi, c0, c1):   # t[y+1, x] (plane 8 = boundary)
                return sut[:, lo + 1 : hi + 1, c0:c1]

            def unc(lo, hi, c0, c1):   # t[y, x+1] (col W = -1 guard)
                return sut[:, lo:hi, c0 + 1 : c1 + 1]

            def mk(tile):
                def f(lo, hi, c0, c1):
                    return tile[:, lo:hi, c0:c1]
                return f

            o2_, ng0_, ng1_, qh_ = mk(o2t), mk(ng0t), mk(ng1t), mk(qht)
            d1_, d2_, db_ = mk(d1t), mk(d2t), mk(dbt)

            def na_(lo, hi, c0, c1):     # -a data (planes 1..8)
                return nat[:, lo + 1 : hi + 1, c0:c1]

            def nb_(lo, hi, c0, c1):     # -b data (cols 1..W)
                return nbt[:, lo:hi, c0 + 1 : c1 + 1]

            def nbpc(lo, hi, c0, c1):    # -b[y, x-1] (col 0 = zero guard)
                return nbt[:, lo:hi, c0:c1]

            def emit(op_name, lo, hi, out_f, a_f, b_f):
                for eng, c0, c1 in ENG:
                    getattr(eng, op_name)(
                        out_f(lo, hi, c0, c1), a_f(lo, hi, c0, c1),
                        b_f(lo, hi, c0, c1))

            # guard-only memsets (interiors are fully overwritten)
            v.memset(sut[:, :, W:WG], -1.0)
            v.memset(sut[:, NP, :], -1.0)   # partition 127 keeps -1; the
            v.memset(nat[:, 0, :], 0.0)     # boundary DMAs rewrite the rest
            v.memset(nbt[:, :, 0:1], 0.0)
            nc.sync.dma_start(out=o2t[:, :, :], in_=o2_v)
            nc.sync.dma_start(out=ng0t[:, :, :], in_=g0_v)
            nc.sync.dma_start(out=ng1t[:, :, :], in_=g1_v)
            nc.sync.dma_start(out=eyet[:, :], in_=eye_d)
            nc.sync.dma_start(out=neyet[:, :], in_=neye_d)

            def dma_ushift():
                # su[p, 8, x] = t[8p+8, x] = su[p+1, 0, x]; row 127 stays -1
                nc.sync.dma_start(
                    out=sut[0 : P - 1, NP, 0:W], in_=sut[1:P, 0, 0:W]
                )

            def dma_ashift():
                # na[p, 0, x] = -a[8p-1] = na[p-1, 8, x]; row 0 stays 0
                nc.sync.dma_start(
                    out=nat[1:P, 0, 0:W], in_=nat[0 : P - 1, NP, 0:W]
                )

            def mm(ps, w, src, start, stop):
                for h in range(W // 512):
                    pe.matmul(
                        ps[:, h * 512 : (h + 1) * 512],
                        w[:, :],
                        src[:, h * 512 : (h + 1) * 512],
                        start=start,
                        stop=stop,
                    )

            for _rep in range(reps):
                if reps > 1:
                    v.memset(sut[:, 0:NP, 0:W], -1.0)
                for lo, hi in [(0, NP // 2), (NP // 2, NP)]:
                    act.activation(
                        sut[:, lo:hi, 0:W], o2t[:, lo:hi, :], AF.Tanh, scale=0.5
                    )
                dma_ushift()

                for it in range(MAXITER):
                    last = it == MAXITER - 1
                    # --- dual TT: d = shifted diffs, t12 = -g * d ---
                    for lo, hi in BLOCKS:
                        emit("tensor_sub", lo, hi, d1_, unr, u_)
                        emit("tensor_mul", lo, hi, d1_, d1_, ng1_)
                        emit("tensor_sub", lo, hi, d2_, unc, u_)
                        emit("tensor_mul", lo, hi, d2_, d2_, ng0_)
                    # --- dual PE accumulate + relu drain (chunk = 1 plane) ---
                    for i in range(NP):
                        ps1 = psp1.tile([P, W], F32, name="ps1")
                        if it > 0:
                            mm(ps1, eyet, qht[:, i, :], True, False)
                        mm(ps1, eyet, d1t[:, i, :], it == 0, False)
                        mm(ps1, eyet, d2t[:, i, :], False, True)
                        act.activation(qht[:, i, :], ps1[:, :], AF.Relu)
                    # --- primal TT ---
                    # na plane-8 row first so the boundary DMA fires early
                    for eng, c0, c1 in ENG:
                        eng.tensor_mul(
                            nat[:, NP, c0:c1],
                            ng1t[:, NP - 1, c0:c1],
                            qht[:, NP - 1, c0:c1],
                        )
                    dma_ashift()
                    for lo, hi in BLOCKS:
                        hi_w = min(hi, NP - 1)
                        if hi_w > lo:
                            emit("tensor_mul", lo, hi_w, na_, ng1_, qh_)
                        emit("tensor_mul", lo, hi, nb_, ng0_, qh_)
                        emit("tensor_sub", lo, hi, db_, nb_, nbpc)
                    # --- primal PE accumulate + tanh drain / output ---
                    for i in range(NP):
                        ps2 = psp2.tile([P, W], F32, name="ps2")
                        mm(ps2, eyet, o2t[:, i, :], True, False)
                        mm(ps2, eyet, nat[:, i + 1, :], False, False)
                        mm(ps2, neyet, nat[:, i, :], False, False)
                        mm(ps2, eyet, dbt[:, i, :], False, True)
                        if last and reps == 1:
                            st = outst[:, i % 4, :]
                            act.activation(st, ps2[:, :], AF.Copy)
                            q = [nc.sync, nc.scalar][i % 2]
                            q.dma_start(out=out_v[:, i, :], in_=st)
                        else:
                            act.activation(
                                sut[:, i, 0:W], ps2[:, :], AF.Tanh, scale=0.5
                            )
                            if i == 0:
                                dma_ushift()

    nc.compile()
    return nc


def kernel(o, vector_field, nabla_w, div_w):
    global LAST_RESULTS
    if "nc" not in _CACHE:
        _CACHE["nc"] = _build()
    nc = _CACHE["nc"]

    o2 = np.ascontiguousarray(
        (2.0 * np.asarray(o, dtype=np.float32)[:, 0]).astype(np.float16)
    )
    vf = np.asarray(vector_field, dtype=np.float32)
    s = np.float32(-1.0 / np.sqrt(2.0))
    ng0 = np.ascontiguousarray((vf[:, :, 0] * s).astype(np.float16))
    ng1 = np.ascontiguousarray((vf[:, :, 1] * s).astype(np.float16))
    eye = np.eye(P, dtype=np.float16)
    neye = -eye

    in_maps = [
        {"o2": o2[b], "ng0": ng0, "ng1": ng1, "eye": eye, "neye": neye}
        for b in range(B)
    ]
    res = bass_utils.run_bass_kernel_spmd(nc, in_maps, core_ids=list(range(B)))
    LAST_RESULTS = res
    return np.stack([r["out"] for r in res.results]).astype(np.float32)


# revision 9
# speedup vs baseline: 3.8750x; 1.0103x over previous
"""Trainium2 Bass kernel for the Chambolle-Pock-style primal/dual stencil loop.

Math (per image, H=W=1024, EPS=0.5, TAU=0.5, 10 iterations):
    u = sigmoid(o/EPS); q = 0
    repeat 10x:
        q  = relu(q - TAU*(vf1*Dy(u) + vf0*Dx(u)))   # forward diffs, zero pad
        Tq = BDy(vf1*q) + BDx(vf0*q)                  # backward diffs, zero pad
        u  = sigmoid((o - Tq)/EPS)
    return (o - Tq)/EPS

Rescaling: with qh = 2*sqrt(2)*q, g = vf/sqrt(2), o2 = 2*o, s = 2(o - Tq),
t = tanh(s/2)  (u = 0.5 + 0.5*t; zero-padding of u becomes (-1)-padding of t):
    qh = relu(qh - g1*(St - t) - g0*(Rt - t))        # S: y+1 shift, R: x+1
    s  = o2 - (a - Sa) - (b - Rb),  a = g1*qh, b = g0*qh   # backward diffs
    t  = tanh(s/2)
and the final output is s.

Three-engine split (all state fp16; validated rel-L2 vs the fp32 jax
reference ~9e-3, under the 2e-2 gate — the error is early relu
decision-boundary noise, not accumulation):
  - 7 tensor-tensor ops/iter run column-split on DVE (cols 0:836, fp16
    2x_1p mode, 0.52 ns/elem) + GpSimd/Pool (cols 836:1024, TT at 0.42 of
    0.83 ns/elem); the split latency-balances the two engines per op:
      d1 = St - t;  t1 = ng1*d1;  d2 = Rt - t;  t2 = ng0*d2
      na = ng1*qh;  nb = ng0*qh;  db = nb - Rnb          (ng = -g, host-side)
  - the remaining 7 adds/iter run on the otherwise-idle TensorEngine as
    +/-identity matmuls accumulating in PSUM (fp32 accumulation - better
    numerics than fp16 adds), one plane ([128,1024] fp32 = 2 PSUM banks)
    per chunk, double-buffered dual + primal pools = all 8 banks:
      psum1 = I*qh + I*t1 + I*t2                 -> ScalarE relu -> qh (fp16)
      psum2 = I*o2 + I*na - I*Sna + I*db         -> ScalarE tanh(x/2) -> t
    (Sna is the plane-shifted read of na, so the "da" diff costs a PE pass
    instead of a DVE op; db keeps the DVE form except plane 7, which goes
    via an extra PE pass pair - that balance measured best.)
  - last iteration: psum2 chunks are Copy-drained to fp16 and DMA'd out on
    alternating queues; the host upcasts to fp32.

Boundary handling: image row y = 8*p + i -> partition p (0..127), plane i
(0..7) in the free dim.  Row shifts are free-dim plane offsets; only the
plane7 -> next-partition boundary crosses partitions, via a tiny SBUF->SBUF
DMA per iteration (t: plane 8 of sut; -a: plane 0 of nat, with a dedicated
plane-7 row multiply emitted first so the DMA fires early).  Column shifts
use guard columns (sut col W = -1; nbt col 0 = 0).  Only guard regions are
memset; interiors are fully overwritten (and the iteration-0 dual skips the
I*qh pass since qh0 = 0, so qht needs no init at all).

Sharding: pure data parallel, one image per NeuronCore (B=8 over 8 cores),
ng0/ng1 broadcast.
"""

import numpy as np

import concourse.bacc as bacc
import concourse.mybir as mybir
from concourse.tile import TileContext
from concourse import bass_utils

F32 = mybir.dt.float32
F16 = mybir.dt.float16
AF = mybir.ActivationFunctionType

B, H, W = 8, 1024, 1024
P = 128          # SBUF partitions
NP = H // P      # planes per partition = 8
WG = W + 2       # plane width incl. one guard column (+1 pad to even)
XS = 836         # DVE handles cols [0, XS), Pool cols [XS, W)
NB = 2           # planes per tensor-tensor block
MAXITER = 10

_CACHE = {}
LAST_RESULTS = None  # BassKernelResults of the most recent run (for test.py)


def _build(reps=1):
    nc = bacc.Bacc("TRN2", target_bir_lowering=False, debug=False)

    o2_d = nc.dram_tensor("o2", [H, W], F16, kind="ExternalInput").ap()
    g0_d = nc.dram_tensor("ng0", [H, W], F16, kind="ExternalInput").ap()
    g1_d = nc.dram_tensor("ng1", [H, W], F16, kind="ExternalInput").ap()
    eye_d = nc.dram_tensor("eye", [P, P], F16, kind="ExternalInput").ap()
    neye_d = nc.dram_tensor("neye", [P, P], F16, kind="ExternalInput").ap()
    out_d = nc.dram_tensor("out", [H, W], F16, kind="ExternalOutput").ap()

    # (H, W) -> (p, i, x) with y = 8*p + i
    o2_v = o2_d.rearrange("(p i) x -> p i x", i=NP)
    g0_v = g0_d.rearrange("(p i) x -> p i x", i=NP)
    g1_v = g1_d.rearrange("(p i) x -> p i x", i=NP)
    out_v = out_d.rearrange("(p i) x -> p i x", i=NP)

    v = nc.vector
    gp = nc.gpsimd
    act = nc.scalar
    pe = nc.tensor

    with TileContext(nc) as tc:
        with (
            tc.tile_pool(name="main", bufs=1) as pool,
            tc.tile_pool(name="ps1", bufs=2, space="PSUM") as psp1,
            tc.tile_pool(name="ps2", bufs=2, space="PSUM") as psp2,
        ):
            o2t = pool.tile([P, NP, W], F16)
            ng0t = pool.tile([P, NP, W], F16)
            ng1t = pool.tile([P, NP, W], F16)
            qht = pool.tile([P, NP, W], F16)
            eyet = pool.tile([P, P], F16)
            neyet = pool.tile([P, P], F16)
            # sut: planes 0..7 = t data (col W = -1 guard for x+1 reads),
            # plane 8 = boundary row t[8p+8, x] (partition 127 stays -1)
            sut = pool.tile([P, NP + 1, WG], F16)
            # d1/d2: diff scratch, overwritten in place by t1/t2
            d1t = pool.tile([P, NP, W], F16)
            d2t = pool.tile([P, NP, W], F16)
            # na: planes 1..8 = -a data, plane 0 = boundary row -a[8p-1, x]
            nat = pool.tile([P, NP + 1, W], F16)
            # nb: cols 1..W = -b data, col 0 = zero guard for x-1 reads
            nbt = pool.tile([P, NP, WG], F16)
            dbt = pool.tile([P, NP, W], F16)
            outst = pool.tile([P, 4, W], F16)   # last-iter drain staging

            ENG = [(v, 0, XS), (gp, XS, W)]
            BLOCKS = [(b * NB, (b + 1) * NB) for b in range(NP // NB)]

            def u_(lo, hi, c0, c1):
                return sut[:, lo:hi, c0:c1]

            def unr(lo, hi, c0, c1):   # t[y+1, x] (plane 8 = boundary)
                return sut[:, lo + 1 : hi + 1, c0:c1]

            def unc(lo, hi, c0, c1):   # t[y, x+1] (col W = -1 guard)
                return sut[:, lo:hi, c0 + 1 : c1 + 1]

            def mk(tile):
                def f(lo, hi, c0, c1):
                    return tile[:, lo:hi, c0:c1]
                return f

            o2_, ng0_, ng1_, qh_ = mk(o2t), mk(ng0t), mk(ng1t), mk(qht)
            d1_, d2_, db_ = mk(d1t), mk(d2t), mk(dbt)

            def na_(lo, hi, c0, c1):     # -a data (planes 1..8)
                return nat[:, lo + 1 : hi + 1, c0:c1]

            def nb_(lo, hi, c0, c1):     # -b data (cols 1..W)
                return nbt[:, lo:hi, c0 + 1 : c1 + 1]

            def nbpc(lo, hi, c0, c1):    # -b[y, x-1] (col 0 = zero guard)
                return nbt[:, lo:hi, c0:c1]

            def emit(op_name, lo, hi, out_f, a_f, b_f):
                for eng, c0, c1 in ENG:
                    getattr(eng, op_name)(
                        out_f(lo, hi, c0, c1), a_f(lo, hi, c0, c1),
                        b_f(lo, hi, c0, c1))

            # guard-only memsets (interiors are fully overwritten)
            v.memset(sut[:, :, W:WG], -1.0)
            v.memset(sut[:, NP, :], -1.0)   # partition 127 keeps -1; the
            v.memset(nat[:, 0, :], 0.0)     # boundary DMAs rewrite the rest
            v.memset(nbt[:, :, 0:1], 0.0)
            nc.sync.dma_start(out=eyet[:, :], in_=eye_d)
            nc.sync.dma_start(out=neyet[:, :], in_=neye_d)
            nc.sync.dma_start(out=o2t[:, :, :], in_=o2_v)
            nc.sync.dma_start(out=ng1t[:, :, :], in_=g1_v)
            nc.sync.dma_start(out=ng0t[:, :, :], in_=g0_v)

            def dma_ushift():
                # su[p, 8, x] = t[8p+8, x] = su[p+1, 0, x]; row 127 stays -1
                nc.sync.dma_start(
                    out=sut[0 : P - 1, NP, 0:W], in_=sut[1:P, 0, 0:W]
                )

            def dma_ashift():
                # na[p, 0, x] = -a[8p-1] = na[p-1, 8, x]; row 0 stays 0
                nc.sync.dma_start(
                    out=nat[1:P, 0, 0:W], in_=nat[0 : P - 1, NP, 0:W]
                )

            def mm(ps, w, src, start, stop):
                for h in range(W // 512):
                    pe.matmul(
                        ps[:, h * 512 : (h + 1) * 512],
                        w[:, :],
                        src[:, h * 512 : (h + 1) * 512],
                        start=start,
                        stop=stop,
                    )

            for _rep in range(reps):
                if reps > 1:
                    v.memset(sut[:, 0:NP, 0:W], -1.0)
                for lo, hi in [(0, NP // 2), (NP // 2, NP)]:
                    act.activation(
                        sut[:, lo:hi, 0:W], o2t[:, lo:hi, :], AF.Tanh, scale=0.5
                    )
                dma_ushift()

                for it in range(MAXITER):
                    last = it == MAXITER - 1
                    # --- dual TT: d = shifted diffs, t12 = -g * d ---
                    for lo, hi in BLOCKS:
                        emit("tensor_sub", lo, hi, d1_, unr, u_)
                        emit("tensor_mul", lo, hi, d1_, d1_, ng1_)
                        emit("tensor_sub", lo, hi, d2_, unc, u_)
                        emit("tensor_mul", lo, hi, d2_, d2_, ng0_)
                    # --- dual PE accumulate + relu drain (chunk = 1 plane) ---
                    for i in range(NP):
                        ps1 = psp1.tile([P, W], F32, name="ps1")
                        if it > 0:
                            mm(ps1, eyet, qht[:, i, :], True, False)
                        mm(ps1, eyet, d1t[:, i, :], it == 0, False)
                        mm(ps1, eyet, d2t[:, i, :], False, True)
                        act.activation(qht[:, i, :], ps1[:, :], AF.Relu)
                    # --- primal TT ---
                    # na plane-8 row first so the boundary DMA fires early
                    for eng, c0, c1 in ENG:
                        eng.tensor_mul(
                            nat[:, NP, c0:c1],
                            ng1t[:, NP - 1, c0:c1],
                            qht[:, NP - 1, c0:c1],
                        )
                    dma_ashift()
                    for lo, hi in BLOCKS:
                        hi_w = min(hi, NP - 1)
                        if hi_w > lo:
                            emit("tensor_mul", lo, hi_w, na_, ng1_, qh_)
                        emit("tensor_mul", lo, hi, nb_, ng0_, qh_)
                        # db plane 7 goes via PE double-pass instead (below)
                        hi_db = min(hi, NP - 1)
                        if hi_db > lo:
                            emit("tensor_sub", lo, hi_db, db_, nb_, nbpc)
                    # --- primal PE accumulate + tanh drain / output ---
                    for i in range(NP):
                        ps2 = psp2.tile([P, W], F32, name="ps2")
                        mm(ps2, eyet, o2t[:, i, :], True, False)
                        mm(ps2, eyet, nat[:, i + 1, :], False, False)
                        if i < NP - 1:
                            mm(ps2, neyet, nat[:, i, :], False, False)
                            mm(ps2, eyet, dbt[:, i, :], False, True)
                        else:
                            mm(ps2, neyet, nat[:, i, :], False, False)
                            mm(ps2, eyet, nbt[:, i, 1 : W + 1], False, False)
                            mm(ps2, neyet, nbt[:, i, 0:W], False, True)
                        if last and reps == 1:
                            st = outst[:, i % 4, :]
                            act.activation(st, ps2[:, :], AF.Copy)
                            q = [nc.sync, nc.scalar][i % 2]
                            q.dma_start(out=out_v[:, i, :], in_=st)
                        else:
                            act.activation(
                                sut[:, i, 0:W], ps2[:, :], AF.Tanh, scale=0.5
                            )
                            if i == 0:
                                dma_ushift()

    nc.compile()
    return nc


def kernel(o, vector_field, nabla_w, div_w):
    global LAST_RESULTS
    if "nc" not in _CACHE:
        _CACHE["nc"] = _build()
    nc = _CACHE["nc"]

    o2 = np.ascontiguousarray(
        (2.0 * np.asarray(o, dtype=np.float32)[:, 0]).astype(np.float16)
    )
    vf = np.asarray(vector_field, dtype=np.float32)
    s = np.float32(-1.0 / np.sqrt(2.0))
    ng0 = np.ascontiguousarray((vf[:, :, 0] * s).astype(np.float16))
    ng1 = np.ascontiguousarray((vf[:, :, 1] * s).astype(np.float16))
    eye = np.eye(P, dtype=np.float16)
    neye = -eye

    in_maps = [
        {"o2": o2[b], "ng0": ng0, "ng1": ng1, "eye": eye, "neye": neye}
        for b in range(B)
    ]
    res = bass_utils.run_bass_kernel_spmd(nc, in_maps, core_ids=list(range(B)))
    LAST_RESULTS = res
    return np.stack([r["out"] for r in res.results]).astype(np.float32)


# revision 10
# speedup vs baseline: 3.8793x; 1.0011x over previous
"""Trainium2 Bass kernel for the Chambolle-Pock-style primal/dual stencil loop.

Math (per image, H=W=1024, EPS=0.5, TAU=0.5, 10 iterations):
    u = sigmoid(o/EPS); q = 0
    repeat 10x:
        q  = relu(q - TAU*(vf1*Dy(u) + vf0*Dx(u)))   # forward diffs, zero pad
        Tq = BDy(vf1*q) + BDx(vf0*q)                  # backward diffs, zero pad
        u  = sigmoid((o - Tq)/EPS)
    return (o - Tq)/EPS

Rescaling: with qh = 2*sqrt(2)*q, g = vf/sqrt(2), o2 = 2*o, s = 2(o - Tq),
t = tanh(s/2)  (u = 0.5 + 0.5*t; zero-padding of u becomes (-1)-padding of t):
    qh = relu(qh - g1*(St - t) - g0*(Rt - t))        # S: y+1 shift, R: x+1
    s  = o2 - (a - Sa) - (b - Rb),  a = g1*qh, b = g0*qh   # backward diffs
    t  = tanh(s/2)
and the final output is s.

Three-engine split (all state fp16; validated rel-L2 vs the fp32 jax
reference ~9e-3, under the 2e-2 gate — the error is early relu
decision-boundary noise, not accumulation):
  - 7 tensor-tensor ops/iter run column-split on DVE (cols 0:838, fp16
    2x_1p mode, 0.52 ns/elem) + GpSimd/Pool (cols 838:1024, TT at 0.42 of
    0.83 ns/elem); the split latency-balances the two engines per op:
      d1 = St - t;  t1 = ng1*d1;  d2 = Rt - t;  t2 = ng0*d2
      na = ng1*qh;  nb = ng0*qh;  db = nb - Rnb          (ng = -g, host-side)
  - the remaining 7 adds/iter run on the otherwise-idle TensorEngine as
    +/-identity matmuls accumulating in PSUM (fp32 accumulation - better
    numerics than fp16 adds), one plane ([128,1024] fp32 = 2 PSUM banks)
    per chunk, double-buffered dual + primal pools = all 8 banks:
      psum1 = I*qh + I*t1 + I*t2                 -> ScalarE relu -> qh (fp16)
      psum2 = I*o2 + I*na - I*Sna + I*db         -> ScalarE tanh(x/2) -> t
    (Sna is the plane-shifted read of na, so the "da" diff costs a PE pass
    instead of a DVE op; db keeps the DVE form except plane 7, which goes
    via an extra PE pass pair - that balance measured best.)
  - last iteration: psum2 chunks are Copy-drained to fp16 and DMA'd out on
    alternating queues; the host upcasts to fp32.

Boundary handling: image row y = 8*p + i -> partition p (0..127), plane i
(0..7) in the free dim.  Row shifts are free-dim plane offsets; only the
plane7 -> next-partition boundary crosses partitions, via a tiny SBUF->SBUF
DMA per iteration (t: plane 8 of sut; -a: plane 0 of nat, with a dedicated
plane-7 row multiply emitted first so the DMA fires early).  Column shifts
use guard columns (sut col W = -1; nbt col 0 = 0).  Only guard regions are
memset; interiors are fully overwritten (and the iteration-0 dual skips the
I*qh pass since qh0 = 0, so qht needs no init at all).

Sharding: pure data parallel, one image per NeuronCore (B=8 over 8 cores),
ng0/ng1 broadcast.
"""

import numpy as np

import concourse.bacc as bacc
import concourse.mybir as mybir
from concourse.tile import TileContext
from concourse import bass_utils

F32 = mybir.dt.float32
F16 = mybir.dt.float16
AF = mybir.ActivationFunctionType

B, H, W = 8, 1024, 1024
P = 128          # SBUF partitions
NP = H // P      # planes per partition = 8
WG = W + 2       # plane width incl. one guard column (+1 pad to even)
XS = 838         # DVE handles cols [0, XS), Pool cols [XS, W)
NB = 2           # planes per tensor-tensor block
MAXITER = 10

_CACHE = {}
LAST_RESULTS = None  # BassKernelResults of the most recent run (for test.py)


def _build(reps=1):
    nc = bacc.Bacc("TRN2", target_bir_lowering=False, debug=False)

    o2_d = nc.dram_tensor("o2", [H, W], F16, kind="ExternalInput").ap()
    g0_d = nc.dram_tensor("ng0", [H, W], F16, kind="ExternalInput").ap()
    g1_d = nc.dram_tensor("ng1", [H, W], F16, kind="ExternalInput").ap()
    eye_d = nc.dram_tensor("eye", [P, P], F16, kind="ExternalInput").ap()
    neye_d = nc.dram_tensor("neye", [P, P], F16, kind="ExternalInput").ap()
    out_d = nc.dram_tensor("out", [H, W], F16, kind="ExternalOutput").ap()

    # (H, W) -> (p, i, x) with y = 8*p + i
    o2_v = o2_d.rearrange("(p i) x -> p i x", i=NP)
    g0_v = g0_d.rearrange("(p i) x -> p i x", i=NP)
    g1_v = g1_d.rearrange("(p i) x -> p i x", i=NP)
    out_v = out_d.rearrange("(p i) x -> p i x", i=NP)

    v = nc.vector
    gp = nc.gpsimd
    act = nc.scalar
    pe = nc.tensor

    with TileContext(nc) as tc:
        with (
            tc.tile_pool(name="main", bufs=1) as pool,
            tc.tile_pool(name="ps1", bufs=2, space="PSUM") as psp1,
            tc.tile_pool(name="ps2", bufs=2, space="PSUM") as psp2,
        ):
            o2t = pool.tile([P, NP, W], F16)
            ng0t = pool.tile([P, NP, W], F16)
            ng1t = pool.tile([P, NP, W], F16)
            qht = pool.tile([P, NP, W], F16)
            eyet = pool.tile([P, P], F16)
            neyet = pool.tile([P, P], F16)
            # sut: planes 0..7 = t data (col W = -1 guard for x+1 reads),
            # plane 8 = boundary row t[8p+8, x] (partition 127 stays -1)
            sut = pool.tile([P, NP + 1, WG], F16)
            # d1/d2: diff scratch, overwritten in place by t1/t2
            d1t = pool.tile([P, NP, W], F16)
            d2t = pool.tile([P, NP, W], F16)
            # na: planes 1..8 = -a data, plane 0 = boundary row -a[8p-1, x]
            nat = pool.tile([P, NP + 1, W], F16)
            # nb: cols 1..W = -b data, col 0 = zero guard for x-1 reads
            nbt = pool.tile([P, NP, WG], F16)
            dbt = pool.tile([P, NP, W], F16)
            outst = pool.tile([P, 4, W], F16)   # last-iter drain staging

            ENG = [(v, 0, XS), (gp, XS, W)]
            BLOCKS = [(b * NB, (b + 1) * NB) for b in range(NP // NB)]

            def u_(lo, hi, c0, c1):
                return sut[:, lo:hi, c0:c1]

            def unr(lo, hi, c0, c1):   # t[y+1, x] (plane 8 = boundary)
                return sut[:, lo + 1 : hi + 1, c0:c1]

            def unc(lo, hi, c0, c1):   # t[y, x+1] (col W = -1 guard)
                return sut[:, lo:hi, c0 + 1 : c1 + 1]

            def mk(tile):
                def f(lo, hi, c0, c1):
                    return tile[:, lo:hi, c0:c1]
                return f

            o2_, ng0_, ng1_, qh_ = mk(o2t), mk(ng0t), mk(ng1t), mk(qht)
            d1_, d2_, db_ = mk(d1t), mk(d2t), mk(dbt)

            def na_(lo, hi, c0, c1):     # -a data (planes 1..8)
                return nat[:, lo + 1 : hi + 1, c0:c1]

            def nb_(lo, hi, c0, c1):     # -b data (cols 1..W)
                return nbt[:, lo:hi, c0 + 1 : c1 + 1]

            def nbpc(lo, hi, c0, c1):    # -b[y, x-1] (col 0 = zero guard)
                return nbt[:, lo:hi, c0:c1]

            def emit(op_name, lo, hi, out_f, a_f, b_f):
                for eng, c0, c1 in ENG:
                    getattr(eng, op_name)(
                        out_f(lo, hi, c0, c1), a_f(lo, hi, c0, c1),
                        b_f(lo, hi, c0, c1))

            # guard-only memsets (interiors are fully overwritten)
            v.memset(sut[:, :, W:WG], -1.0)
            v.memset(sut[:, NP, :], -1.0)   # partition 127 keeps -1; the
            v.memset(nat[:, 0, :], 0.0)     # boundary DMAs rewrite the rest
            v.memset(nbt[:, :, 0:1], 0.0)
            nc.sync.dma_start(out=eyet[:, :], in_=eye_d)
            nc.sync.dma_start(out=neyet[:, :], in_=neye_d)
            nc.sync.dma_start(out=o2t[:, 0 : NP // 2, :],
                              in_=o2_v[:, 0 : NP // 2, :])
            nc.scalar.dma_start(out=o2t[:, NP // 2 : NP, :],
                                in_=o2_v[:, NP // 2 : NP, :])
            nc.sync.dma_start(out=ng1t[:, :, :], in_=g1_v)
            nc.sync.dma_start(out=ng0t[:, :, :], in_=g0_v)

            def dma_ushift():
                # su[p, 8, x] = t[8p+8, x] = su[p+1, 0, x]; row 127 stays -1
                nc.sync.dma_start(
                    out=sut[0 : P - 1, NP, 0:W], in_=sut[1:P, 0, 0:W]
                )

            def dma_ashift():
                # na[p, 0, x] = -a[8p-1] = na[p-1, 8, x]; row 0 stays 0
                nc.sync.dma_start(
                    out=nat[1:P, 0, 0:W], in_=nat[0 : P - 1, NP, 0:W]
                )

            def mm(ps, w, src, start, stop):
                for h in range(W // 512):
                    pe.matmul(
                        ps[:, h * 512 : (h + 1) * 512],
                        w[:, :],
                        src[:, h * 512 : (h + 1) * 512],
                        start=start,
                        stop=stop,
                    )

            for _rep in range(reps):
                if reps > 1:
                    v.memset(sut[:, 0:NP, 0:W], -1.0)
                for lo, hi in [(0, NP // 2), (NP // 2, NP)]:
                    act.activation(
                        sut[:, lo:hi, 0:W], o2t[:, lo:hi, :], AF.Tanh, scale=0.5
                    )
                dma_ushift()

                for it in range(MAXITER):
                    last = it == MAXITER - 1
                    # --- dual TT: d = shifted diffs, t12 = -g * d ---
                    for lo, hi in BLOCKS:
                        emit("tensor_sub", lo, hi, d1_, unr, u_)
                        emit("tensor_mul", lo, hi, d1_, d1_, ng1_)
                        emit("tensor_sub", lo, hi, d2_, unc, u_)
                        emit("tensor_mul", lo, hi, d2_, d2_, ng0_)
                    # --- dual PE accumulate + relu drain (chunk = 1 plane) ---
                    for i in range(NP):
                        ps1 = psp1.tile([P, W], F32, name="ps1")
                        if it > 0:
                            mm(ps1, eyet, qht[:, i, :], True, False)
                        mm(ps1, eyet, d1t[:, i, :], it == 0, False)
                        mm(ps1, eyet, d2t[:, i, :], False, True)
                        act.activation(qht[:, i, :], ps1[:, :], AF.Relu)
                    # --- primal TT ---
                    # na plane-8 row first so the boundary DMA fires early
                    for eng, c0, c1 in ENG:
                        eng.tensor_mul(
                            nat[:, NP, c0:c1],
                            ng1t[:, NP - 1, c0:c1],
                            qht[:, NP - 1, c0:c1],
                        )
                    dma_ashift()
                    for lo, hi in BLOCKS:
                        hi_w = min(hi, NP - 1)
                        if hi_w > lo:
                            emit("tensor_mul", lo, hi_w, na_, ng1_, qh_)
                        emit("tensor_mul", lo, hi, nb_, ng0_, qh_)
                        # db plane 7 goes via PE double-pass instead (below)
                        hi_db = min(hi, NP - 1)
                        if hi_db > lo:
                            emit("tensor_sub", lo, hi_db, db_, nb_, nbpc)
                    # --- primal PE accumulate + tanh drain / output ---
                    for i in range(NP):
                        ps2 = psp2.tile([P, W], F32, name="ps2")
                        mm(ps2, eyet, o2t[:, i, :], True, False)
                        mm(ps2, eyet, nat[:, i + 1, :], False, False)
                        if i < NP - 1:
                            mm(ps2, neyet, nat[:, i, :], False, False)
                            mm(ps2, eyet, dbt[:, i, :], False, True)
                        else:
                            mm(ps2, neyet, nat[:, i, :], False, False)
                            mm(ps2, eyet, nbt[:, i, 1 : W + 1], False, False)
                            mm(ps2, neyet, nbt[:, i, 0:W], False, True)
                        if last and reps == 1:
                            st = outst[:, i % 4, :]
                            act.activation(st, ps2[:, :], AF.Copy)
                            q = [nc.sync, nc.scalar][i % 2]
                            q.dma_start(out=out_v[:, i, :], in_=st)
                        else:
                            act.activation(
                                sut[:, i, 0:W], ps2[:, :], AF.Tanh, scale=0.5
                            )
                            if i == 0:
                                dma_ushift()

    nc.compile()
    return nc


def kernel(o, vector_field, nabla_w, div_w):
    global LAST_RESULTS
    if "nc" not in _CACHE:
        _CACHE["nc"] = _build()
    nc = _CACHE["nc"]

    o2 = np.ascontiguousarray(
        (2.0 * np.asarray(o, dtype=np.float32)[:, 0]).astype(np.float16)
    )
    vf = np.asarray(vector_field, dtype=np.float32)
    s = np.float32(-1.0 / np.sqrt(2.0))
    ng0 = np.ascontiguousarray((vf[:, :, 0] * s).astype(np.float16))
    ng1 = np.ascontiguousarray((vf[:, :, 1] * s).astype(np.float16))
    eye = np.eye(P, dtype=np.float16)
    neye = -eye

    in_maps = [
        {"o2": o2[b], "ng0": ng0, "ng1": ng1, "eye": eye, "neye": neye}
        for b in range(B)
    ]
    res = bass_utils.run_bass_kernel_spmd(nc, in_maps, core_ids=list(range(B)))
    LAST_RESULTS = res
    return np.stack([r["out"] for r in res.results]).astype(np.float32)


# revision 11
# speedup vs baseline: 3.8802x; 1.0002x over previous
"""Trainium2 Bass kernel for the Chambolle-Pock-style primal/dual stencil loop.

Math (per image, H=W=1024, EPS=0.5, TAU=0.5, 10 iterations):
    u = sigmoid(o/EPS); q = 0
    repeat 10x:
        q  = relu(q - TAU*(vf1*Dy(u) + vf0*Dx(u)))   # forward diffs, zero pad
        Tq = BDy(vf1*q) + BDx(vf0*q)                  # backward diffs, zero pad
        u  = sigmoid((o - Tq)/EPS)
    return (o - Tq)/EPS

Rescaling: with qh = 2*sqrt(2)*q, g = vf/sqrt(2), o2 = 2*o, s = 2(o - Tq),
t = tanh(s/2)  (u = 0.5 + 0.5*t; zero-padding of u becomes (-1)-padding of t):
    qh = relu(qh - g1*(St - t) - g0*(Rt - t))        # S: y+1 shift, R: x+1
    s  = o2 - (a - Sa) - (b - Rb),  a = g1*qh, b = g0*qh   # backward diffs
    t  = tanh(s/2)
and the final output is s.

Three-engine split (all state fp16; validated rel-L2 vs the fp32 jax
reference ~9e-3, under the 2e-2 gate — the error is early relu
decision-boundary noise, not accumulation):
  - 7 tensor-tensor ops/iter run column-split on DVE (cols 0:838, fp16
    2x_1p mode, 0.52 ns/elem) + GpSimd/Pool (cols 838:1024, TT at 0.42 of
    0.83 ns/elem); the split latency-balances the two engines per op:
      d1 = St - t;  t1 = ng1*d1;  d2 = Rt - t;  t2 = ng0*d2
      na = ng1*qh;  nb = ng0*qh;  db = nb - Rnb          (ng = -g, host-side)
  - the remaining 7 adds/iter run on the otherwise-idle TensorEngine as
    +/-identity matmuls accumulating in PSUM (fp32 accumulation - better
    numerics than fp16 adds), one plane ([128,1024] fp32 = 2 PSUM banks)
    per chunk, double-buffered dual + primal pools = all 8 banks:
      psum1 = I*qh + I*t1 + I*t2                 -> ScalarE relu -> qh (fp16)
      psum2 = I*o2 + I*na - I*Sna + I*db         -> ScalarE tanh(x/2) -> t
    (Sna is the plane-shifted read of na, so the "da" diff costs a PE pass
    instead of a DVE op; db keeps the DVE form except plane 7, which goes
    via an extra PE pass pair - that balance measured best.)
  - last iteration: psum2 chunks are Copy-drained to fp16 and DMA'd out on
    the SP queue (a scalar-queue DMA issue would block ActE's sequencer
    between Copy drains); the host upcasts to fp32.

Boundary handling: image row y = 8*p + i -> partition p (0..127), plane i
(0..7) in the free dim.  Row shifts are free-dim plane offsets; only the
plane7 -> next-partition boundary crosses partitions, via a tiny SBUF->SBUF
DMA per iteration (t: plane 8 of sut; -a: plane 0 of nat, with a dedicated
plane-7 row multiply emitted first so the DMA fires early).  Column shifts
use guard columns (sut col W = -1; nbt col 0 = 0).  Only guard regions are
memset; interiors are fully overwritten (and the iteration-0 dual skips the
I*qh pass since qh0 = 0, so qht needs no init at all).

Sharding: pure data parallel, one image per NeuronCore (B=8 over 8 cores),
ng0/ng1 broadcast.
"""

import numpy as np

import concourse.bacc as bacc
import concourse.mybir as mybir
from concourse.tile import TileContext
from concourse import bass_utils

F32 = mybir.dt.float32
F16 = mybir.dt.float16
AF = mybir.ActivationFunctionType

B, H, W = 8, 1024, 1024
P = 128          # SBUF partitions
NP = H // P      # planes per partition = 8
WG = W + 2       # plane width incl. one guard column (+1 pad to even)
XS = 838         # DVE handles cols [0, XS), Pool cols [XS, W)
NB = 2           # planes per tensor-tensor block
MAXITER = 10

_CACHE = {}
LAST_RESULTS = None  # BassKernelResults of the most recent run (for test.py)


def _build(reps=1):
    nc = bacc.Bacc("TRN2", target_bir_lowering=False, debug=False)

    o2_d = nc.dram_tensor("o2", [H, W], F16, kind="ExternalInput").ap()
    g0_d = nc.dram_tensor("ng0", [H, W], F16, kind="ExternalInput").ap()
    g1_d = nc.dram_tensor("ng1", [H, W], F16, kind="ExternalInput").ap()
    eye_d = nc.dram_tensor("eye", [P, P], F16, kind="ExternalInput").ap()
    neye_d = nc.dram_tensor("neye", [P, P], F16, kind="ExternalInput").ap()
    out_d = nc.dram_tensor("out", [H, W], F16, kind="ExternalOutput").ap()

    # (H, W) -> (p, i, x) with y = 8*p + i
    o2_v = o2_d.rearrange("(p i) x -> p i x", i=NP)
    g0_v = g0_d.rearrange("(p i) x -> p i x", i=NP)
    g1_v = g1_d.rearrange("(p i) x -> p i x", i=NP)
    out_v = out_d.rearrange("(p i) x -> p i x", i=NP)

    v = nc.vector
    gp = nc.gpsimd
    act = nc.scalar
    pe = nc.tensor

    with TileContext(nc) as tc:
        with (
            tc.tile_pool(name="main", bufs=1) as pool,
            tc.tile_pool(name="ps1", bufs=2, space="PSUM") as psp1,
            tc.tile_pool(name="ps2", bufs=2, space="PSUM") as psp2,
        ):
            o2t = pool.tile([P, NP, W], F16)
            ng0t = pool.tile([P, NP, W], F16)
            ng1t = pool.tile([P, NP, W], F16)
            qht = pool.tile([P, NP, W], F16)
            eyet = pool.tile([P, P], F16)
            neyet = pool.tile([P, P], F16)
            # sut: planes 0..7 = t data (col W = -1 guard for x+1 reads),
            # plane 8 = boundary row t[8p+8, x] (partition 127 stays -1)
            sut = pool.tile([P, NP + 1, WG], F16)
            # d1/d2: diff scratch, overwritten in place by t1/t2
            d1t = pool.tile([P, NP, W], F16)
            d2t = pool.tile([P, NP, W], F16)
            # na: planes 1..8 = -a data, plane 0 = boundary row -a[8p-1, x]
            nat = pool.tile([P, NP + 1, W], F16)
            # nb: cols 1..W = -b data, col 0 = zero guard for x-1 reads
            nbt = pool.tile([P, NP, WG], F16)
            dbt = pool.tile([P, NP, W], F16)
            outst = pool.tile([P, 4, W], F16)   # last-iter drain staging

            ENG = [(v, 0, XS), (gp, XS, W)]
            BLOCKS = [(b * NB, (b + 1) * NB) for b in range(NP // NB)]

            def u_(lo, hi, c0, c1):
                return sut[:, lo:hi, c0:c1]

            def unr(lo, hi, c0, c1):   # t[y+1, x] (plane 8 = boundary)
                return sut[:, lo + 1 : hi + 1, c0:c1]

            def unc(lo, hi, c0, c1):   # t[y, x+1] (col W = -1 guard)
                return sut[:, lo:hi, c0 + 1 : c1 + 1]

            def mk(tile):
                def f(lo, hi, c0, c1):
                    return tile[:, lo:hi, c0:c1]
                return f

            o2_, ng0_, ng1_, qh_ = mk(o2t), mk(ng0t), mk(ng1t), mk(qht)
            d1_, d2_, db_ = mk(d1t), mk(d2t), mk(dbt)

            def na_(lo, hi, c0, c1):     # -a data (planes 1..8)
                return nat[:, lo + 1 : hi + 1, c0:c1]

            def nb_(lo, hi, c0, c1):     # -b data (cols 1..W)
                return nbt[:, lo:hi, c0 + 1 : c1 + 1]

            def nbpc(lo, hi, c0, c1):    # -b[y, x-1] (col 0 = zero guard)
                return nbt[:, lo:hi, c0:c1]

            def emit(op_name, lo, hi, out_f, a_f, b_f):
                for eng, c0, c1 in ENG:
                    getattr(eng, op_name)(
                        out_f(lo, hi, c0, c1), a_f(lo, hi, c0, c1),
                        b_f(lo, hi, c0, c1))

            # guard-only memsets (interiors are fully overwritten)
            v.memset(sut[:, :, W:WG], -1.0)
            v.memset(sut[:, NP, :], -1.0)   # partition 127 keeps -1; the
            v.memset(nat[:, 0, :], 0.0)     # boundary DMAs rewrite the rest
            v.memset(nbt[:, :, 0:1], 0.0)
            nc.sync.dma_start(out=eyet[:, :], in_=eye_d)
            nc.sync.dma_start(out=neyet[:, :], in_=neye_d)
            nc.sync.dma_start(out=o2t[:, 0 : NP // 2, :],
                              in_=o2_v[:, 0 : NP // 2, :])
            nc.scalar.dma_start(out=o2t[:, NP // 2 : NP, :],
                                in_=o2_v[:, NP // 2 : NP, :])
            nc.sync.dma_start(out=ng1t[:, :, :], in_=g1_v)
            nc.sync.dma_start(out=ng0t[:, :, :], in_=g0_v)

            def dma_ushift():
                # su[p, 8, x] = t[8p+8, x] = su[p+1, 0, x]; row 127 stays -1
                nc.sync.dma_start(
                    out=sut[0 : P - 1, NP, 0:W], in_=sut[1:P, 0, 0:W]
                )

            def dma_ashift():
                # na[p, 0, x] = -a[8p-1] = na[p-1, 8, x]; row 0 stays 0
                nc.sync.dma_start(
                    out=nat[1:P, 0, 0:W], in_=nat[0 : P - 1, NP, 0:W]
                )

            def mm(ps, w, src, start, stop):
                for h in range(W // 512):
                    pe.matmul(
                        ps[:, h * 512 : (h + 1) * 512],
                        w[:, :],
                        src[:, h * 512 : (h + 1) * 512],
                        start=start,
                        stop=stop,
                    )

            for _rep in range(reps):
                if reps > 1:
                    v.memset(sut[:, 0:NP, 0:W], -1.0)
                for lo, hi in [(0, NP // 2), (NP // 2, NP)]:
                    act.activation(
                        sut[:, lo:hi, 0:W], o2t[:, lo:hi, :], AF.Tanh, scale=0.5
                    )
                dma_ushift()

                for it in range(MAXITER):
                    last = it == MAXITER - 1
                    # --- dual TT: d = shifted diffs, t12 = -g * d ---
                    for lo, hi in BLOCKS:
                        emit("tensor_sub", lo, hi, d1_, unr, u_)
                        emit("tensor_mul", lo, hi, d1_, d1_, ng1_)
                        emit("tensor_sub", lo, hi, d2_, unc, u_)
                        emit("tensor_mul", lo, hi, d2_, d2_, ng0_)
                    # --- dual PE accumulate + relu drain (chunk = 1 plane) ---
                    for i in range(NP):
                        ps1 = psp1.tile([P, W], F32, name="ps1")
                        if it > 0:
                            mm(ps1, eyet, qht[:, i, :], True, False)
                        mm(ps1, eyet, d1t[:, i, :], it == 0, False)
                        mm(ps1, eyet, d2t[:, i, :], False, True)
                        act.activation(qht[:, i, :], ps1[:, :], AF.Relu)
                    # --- primal TT ---
                    # na plane-8 row first so the boundary DMA fires early
                    for eng, c0, c1 in ENG:
                        eng.tensor_mul(
                            nat[:, NP, c0:c1],
                            ng1t[:, NP - 1, c0:c1],
                            qht[:, NP - 1, c0:c1],
                        )
                    dma_ashift()
                    for lo, hi in BLOCKS:
                        hi_w = min(hi, NP - 1)
                        if hi_w > lo:
                            emit("tensor_mul", lo, hi_w, na_, ng1_, qh_)
                        emit("tensor_mul", lo, hi, nb_, ng0_, qh_)
                        # db plane 7 goes via PE double-pass instead (below)
                        hi_db = min(hi, NP - 1)
                        if hi_db > lo:
                            emit("tensor_sub", lo, hi_db, db_, nb_, nbpc)
                    # --- primal PE accumulate + tanh drain / output ---
                    for i in range(NP):
                        ps2 = psp2.tile([P, W], F32, name="ps2")
                        mm(ps2, eyet, o2t[:, i, :], True, False)
                        mm(ps2, eyet, nat[:, i + 1, :], False, False)
                        if i < NP - 1:
                            mm(ps2, neyet, nat[:, i, :], False, False)
                            mm(ps2, eyet, dbt[:, i, :], False, True)
                        else:
                            mm(ps2, neyet, nat[:, i, :], False, False)
                            mm(ps2, eyet, nbt[:, i, 1 : W + 1], False, False)
                            mm(ps2, neyet, nbt[:, i, 0:W], False, True)
                        if last and reps == 1:
                            st = outst[:, i % 4, :]
                            act.activation(st, ps2[:, :], AF.Copy)
                            nc.sync.dma_start(out=out_v[:, i, :], in_=st)
                        else:
                            act.activation(
                                sut[:, i, 0:W], ps2[:, :], AF.Tanh, scale=0.5
                            )
                            if i == 0:
                                dma_ushift()

    nc.compile()
    return nc


def kernel(o, vector_field, nabla_w, div_w):
    global LAST_RESULTS
    if "nc" not in _CACHE:
        _CACHE["nc"] = _build()
    nc = _CACHE["nc"]

    o2 = np.ascontiguousarray(
        (2.0 * np.asarray(o, dtype=np.float32)[:, 0]).astype(np.float16)
    )
    vf = np.asarray(vector_field, dtype=np.float32)
    s = np.float32(-1.0 / np.sqrt(2.0))
    ng0 = np.ascontiguousarray((vf[:, :, 0] * s).astype(np.float16))
    ng1 = np.ascontiguousarray((vf[:, :, 1] * s).astype(np.float16))
    eye = np.eye(P, dtype=np.float16)
    neye = -eye

    in_maps = [
        {"o2": o2[b], "ng0": ng0, "ng1": ng1, "eye": eye, "neye": neye}
        for b in range(B)
    ]
    res = bass_utils.run_bass_kernel_spmd(nc, in_maps, core_ids=list(range(B)))
    LAST_RESULTS = res
    return np.stack([r["out"] for r in res.results]).astype(np.float32)
